# revision 20
# baseline (speedup 1.0000x reference)
"""Trainium2 Bass kernel for nn_DeliveryEventEncoder.

Strategy: pure data parallel across 8 NeuronCores (4 buildings = 128 units
per core). Activations are kept in feature-major layout [feat(128 part),
seq(256 free)] so every weight matmul streams 256 columns; matmul inputs
are bf16 (1 cyc/row on PE), accumulation is fp32 in PSUM, LayerNorm
stats/softmax denominators are fp32. The ragged key mask folds into v and
the softmax denominator (no masking of exp tiles); the query mask folds
into LN2's rstd so the ragged sum-pool is a plain ones-matmul.

The per-unit work is emitted in two phases per group of 8 units: phase A
(everything through softmax exp — act-func-set "exp") for all 8 units,
then phase B (LayerNorm sqrt, relu, copies — act-func-set "sqrt") for all
8. The ACT PWP table reload costs 1.28us, so alternating exp/sqrt per
unit would burn ~330us/core; grouping drops it to 2 reloads per 8 units.
"""

import os
import numpy as np
import ml_dtypes

import concourse.bass as bass
import concourse.bacc as bacc_mod
import concourse.mybir as mybir
import concourse.tile as tile
from concourse.bass_utils import run_bass_kernel_spmd
from concourse.masks import make_identity

F32 = mybir.dt.float32
BF16 = mybir.dt.bfloat16
AF = mybir.ActivationFunctionType
ALU = mybir.AluOpType
NPBF = ml_dtypes.bfloat16

B, U, L, DSEQ, H, DOUT = 32, 32, 256, 5, 128, 128
TODV, TODD, AGGD, UNITD = 5, 3, 7, 16
NCORES = 8
BPC = B // NCORES          # buildings per core
NU = BPC * U               # units per core (128)
GRP = int(os.environ.get('KGRP', '8'))   # units per X-group DMA / phase block
NGRP = NU // GRP
CSCALE = 1.0 / np.sqrt(H)
EPS = 1e-5


def build_nc(wts):
    """Build the SPMD Bass module. `wts`: numpy bf16 weight arrays (already
    transposed for lhsT use), baked in as inline consts."""
    nc = bacc_mod.Bacc()

    x_in = nc.dram_tensor("xg", [NGRP, DSEQ, GRP * L], BF16, kind="ExternalInput")
    m01_in = nc.dram_tensor("m01", [128, NU * 2], F32, kind="ExternalInput")
    m01b_in = nc.dram_tensor("m01b", [128, NU * 2], BF16, kind="ExternalInput")
    s_in = nc.dram_tensor("S", [NU, BPC], BF16, kind="ExternalInput")
    tail_in = nc.dram_tensor("tail", [AGGD + TODD, BPC], BF16, kind="ExternalInput")
    out_t = nc.dram_tensor("outT", [DOUT, BPC], F32, kind="ExternalOutput")

    dW = {k: nc.inline_tensor(v, name=k) for k, v in wts.items()}

    cfg = dict(xp=2, wk=3, nt=3, sm=8, pp=2 * GRP + 1, ps=3, pn=3, pc=1, pa=1)
    for kv in os.environ.get("KPOOLS", "").split(","):
        if kv:
            k_, v_ = kv.split("=")
            cfg[k_] = int(v_)

    with tile.TileContext(nc) as tc:
        with (
            tc.tile_pool(name="singles", bufs=1) as singles,
            tc.tile_pool(name="xpool", bufs=cfg["xp"]) as xpool,
            tc.tile_pool(name="work", bufs=cfg["wk"]) as work,
            tc.tile_pool(name="nat", bufs=cfg["nt"]) as natp,
            tc.tile_pool(name="small", bufs=cfg["sm"]) as small,
            tc.tile_pool(name="pipe", bufs=cfg["pp"]) as pipe,
            tc.tile_pool(name="pipe2", bufs=2 * cfg["pp"]) as pipe2,
            tc.tile_pool(name="ps", bufs=cfg["ps"], space="PSUM") as ps,
            tc.tile_pool(name="psn", bufs=cfg["pn"], space="PSUM") as psn,
            tc.tile_pool(name="pcol", bufs=cfg["pc"], space="PSUM") as pcol,
            tc.tile_pool(name="pacc", bufs=cfg["pa"], space="PSUM") as pacc,
        ):
            # ---- constants into SBUF ----
            def load_w(name, p, f):
                t = singles.tile([p, f], BF16, tag=name)
                nc.gpsimd.dma_start(out=t, in_=dW[name][:, :])
                return t

            w_in = load_w("w_inT", DSEQ, H)
            w_g = load_w("w_gT", H, H)
            w_v = load_w("w_vT", H, H)
            w_o = load_w("w_oT", H, H)
            w_f1 = load_w("w_f1T", H, H)
            w_f2 = load_w("w_f2T", H, H)
            w_u = load_w("w_uT", H, UNITD)
            w_c1 = load_w("w_c1T", UNITD + AGGD + TODD, H)
            w_c2 = load_w("w_c2T", H, DOUT)

            ident = singles.tile([128, 128], F32, tag="ident")
            make_identity(nc, ident)
            ones_b = singles.tile([128, 1], BF16, tag="ones")
            nc.vector.memset(ones_b, 1.0)
            eps_col = singles.tile([128, 1], F32, tag="eps")
            nc.vector.memset(eps_col, EPS)

            s_sb = singles.tile([NU, BPC], BF16, tag="S")
            nc.gpsimd.dma_start(out=s_sb, in_=s_in[:, :])
            m01_all = singles.tile([128, NU * 2], F32, tag="m01")
            nc.gpsimd.dma_start(out=m01_all, in_=m01_in[:, :])
            m01b = singles.tile([128, NU * 2], BF16, tag="m01b")
            nc.gpsimd.dma_start(out=m01b, in_=m01b_in[:, :])

            pooled = singles.tile([H, NU], BF16, tag="pooled")

            def phase_a(xs, kk, u):
                """emb/q/k/v/scores/exp for one unit (act set: exp)."""
                xu = xs[:, kk * L:(kk + 1) * L]

                emb_ps = ps.tile([H, L], F32, tag="ps")
                nc.tensor.matmul(emb_ps, w_in, xu, start=True, stop=True)
                embT = work.tile([H, L], BF16, tag="embT")
                (nc.vector if os.environ.get("KCPE") else nc.any).tensor_copy(embT, emb_ps)

                embn = []
                for lt in range(2):
                    en_ps = psn.tile([128, H], F32, tag="psn")
                    nc.tensor.matmul(
                        en_ps, xu[:, lt * 128:(lt + 1) * 128], w_in,
                        start=True, stop=True)
                    en = pipe2.tile([128, H], F32, tag="embn")
                    nc.any.tensor_copy(en, en_ps)
                    embn.append(en)

                y_ps = ps.tile([H, L], F32, tag="ps")
                nc.tensor.matmul(y_ps, w_g, embT, start=True, stop=True)
                yT = work.tile([H, L], BF16, tag="yT")
                (nc.vector if os.environ.get("KCPE") else nc.any).tensor_copy(yT, y_ps)

                v_s = []
                for mt in range(2):
                    v_ps = psn.tile([128, H], F32, tag="psn")
                    nc.tensor.matmul(
                        v_ps, embT[:, mt * 128:(mt + 1) * 128], w_v,
                        start=True, stop=True)
                    vs = pipe.tile([128, H], BF16, tag=f"v{mt}")
                    # key mask folds into v (per-partition scale)
                    if os.environ.get("KVMASK") == "dve":
                        nc.vector.tensor_scalar_mul(
                            out=vs, in0=v_ps,
                            scalar1=m01_all[:, 2 * u + mt:2 * u + mt + 1])
                    else:
                        nc.scalar.activation(
                            out=vs, in_=v_ps, func=AF.Copy, bias=0.0,
                            scale=m01_all[:, 2 * u + mt:2 * u + mt + 1])
                    v_s.append(vs)

                exp_s = []
                for mt in range(2):
                    sc_ps = ps.tile([128, L], F32, tag="ps")
                    nc.tensor.matmul(
                        sc_ps, embT[:, mt * 128:(mt + 1) * 128], yT,
                        start=True, stop=True)
                    es = pipe.tile([128, L], BF16, tag=f"exp{mt}")
                    nc.scalar.activation(
                        out=es, in_=sc_ps, func=AF.Exp, bias=0.0, scale=CSCALE)
                    exp_s.append(es)
                return dict(u=u, embn=embn, v_s=v_s, exp_s=exp_s)

            def phase_b(st):
                """attention apply + LNs + FFN + pool (act set: sqrt)."""
                u, embn, v_s, exp_s = st["u"], st["embn"], st["v_s"], st["exp_s"]

                rec = []
                for lt in range(2):
                    den_ps = pcol.tile([128, 1], F32, tag="pcol")
                    for mt in range(2):
                        nc.tensor.matmul(
                            den_ps, exp_s[mt][:, lt * 128:(lt + 1) * 128],
                            m01b[:, 2 * u + mt:2 * u + mt + 1],
                            start=(mt == 0), stop=(mt == 1))
                    rc = small.tile([128, 1], F32, tag="rec")
                    nc.vector.reciprocal(rc, den_ps)
                    rec.append(rc)

                ao_ps = ps.tile([H, L], F32, tag="ps")
                for mt in range(2):
                    nc.tensor.matmul(ao_ps, v_s[mt], exp_s[mt],
                                     start=(mt == 0), stop=(mt == 1))
                aoT = work.tile([H, L], BF16, tag="aoT")
                nc.any.tensor_copy(aoT, ao_ps)

                x1_nat = []
                for lt in range(2):
                    sl = slice(lt * 128, (lt + 1) * 128)
                    pon_ps = psn.tile([128, H], F32, tag="psn")
                    nc.tensor.matmul(pon_ps, aoT[:, sl], w_o,
                                     start=True, stop=True)
                    x1in = natp.tile([128, H], F32, tag="x1in")
                    s1 = small.tile([128, 1], F32, tag="s1")
                    nc.vector.scalar_tensor_tensor(
                        out=x1in, in0=pon_ps, scalar=rec[lt], in1=embn[lt],
                        op0=ALU.mult, op1=ALU.add, accum_out=s1)
                    sq = natp.tile([128, H], BF16, tag="sq")
                    q1 = small.tile([128, 1], F32, tag="q1")
                    nc.scalar.activation(out=sq, in_=x1in, func=AF.Square,
                                         bias=0.0, scale=1.0, accum_out=q1)
                    mean = small.tile([128, 1], F32, tag="mean")
                    nc.vector.tensor_scalar(
                        out=mean, in0=s1, scalar1=1.0 / H, scalar2=None,
                        op0=ALU.mult)
                    msq = small.tile([128, 1], F32, tag="msq")
                    nc.vector.tensor_tensor(
                        out=msq, in0=mean, in1=mean, op=ALU.mult)
                    var = small.tile([128, 1], F32, tag="var")
                    nc.vector.scalar_tensor_tensor(
                        out=var, in0=q1, scalar=1.0 / H, in1=msq,
                        op0=ALU.mult, op1=ALU.subtract)
                    sd = small.tile([128, 1], F32, tag="sd")
                    nc.scalar.activation(out=sd, in_=var, func=AF.Sqrt,
                                         bias=eps_col, scale=1.0)
                    rs = small.tile([128, 1], F32, tag="rs")
                    nc.vector.reciprocal(rs, sd)
                    x1 = natp.tile([128, H], F32, tag="x1")
                    nc.vector.tensor_scalar(
                        out=x1, in0=x1in, scalar1=mean, scalar2=rs,
                        op0=ALU.subtract, op1=ALU.mult)
                    x1_nat.append(x1)

                x1T = work.tile([H, L], BF16, tag="x1T")
                for lt in range(2):
                    x1t_ps = psn.tile([128, H], F32, tag="psn")
                    nc.tensor.transpose(x1t_ps, x1_nat[lt], ident)
                    nc.any.tensor_copy(x1T[:, lt * 128:(lt + 1) * 128], x1t_ps)

                f1_ps = ps.tile([H, L], F32, tag="ps")
                nc.tensor.matmul(f1_ps, w_f1, x1T, start=True, stop=True)
                f1 = work.tile([H, L], BF16, tag="f1")
                nc.scalar.activation(out=f1, in_=f1_ps, func=AF.Relu,
                                     bias=0.0, scale=1.0)

                pool_ps = pacc.tile([H, 1], F32, tag="pacc")
                for lt in range(2):
                    sl = slice(lt * 128, (lt + 1) * 128)
                    f2n_ps = psn.tile([128, H], F32, tag="psn")
                    nc.tensor.matmul(f2n_ps, f1[:, sl], w_f2,
                                     start=True, stop=True)
                    x2in = natp.tile([128, H], F32, tag="x2in")
                    s2 = small.tile([128, 1], F32, tag="s1")
                    nc.vector.scalar_tensor_tensor(
                        out=x2in, in0=f2n_ps, scalar=1.0, in1=x1_nat[lt],
                        op0=ALU.mult, op1=ALU.add, accum_out=s2)
                    sq2 = natp.tile([128, H], BF16, tag="sq")
                    q2 = small.tile([128, 1], F32, tag="q1")
                    nc.scalar.activation(out=sq2, in_=x2in, func=AF.Square,
                                         bias=0.0, scale=1.0, accum_out=q2)
                    mean2 = small.tile([128, 1], F32, tag="mean")
                    nc.vector.tensor_scalar(
                        out=mean2, in0=s2, scalar1=1.0 / H, scalar2=None,
                        op0=ALU.mult)
                    msq2 = small.tile([128, 1], F32, tag="msq")
                    nc.vector.tensor_tensor(
                        out=msq2, in0=mean2, in1=mean2, op=ALU.mult)
                    var2 = small.tile([128, 1], F32, tag="var")
                    nc.vector.scalar_tensor_tensor(
                        out=var2, in0=q2, scalar=1.0 / H, in1=msq2,
                        op0=ALU.mult, op1=ALU.subtract)
                    sd2 = small.tile([128, 1], F32, tag="sd")
                    nc.scalar.activation(out=sd2, in_=var2, func=AF.Sqrt,
                                         bias=eps_col, scale=1.0)
                    rs2 = small.tile([128, 1], F32, tag="rs")
                    nc.vector.reciprocal(rs2, sd2)
                    rs2m = small.tile([128, 1], F32, tag="rs2m")
                    nc.vector.tensor_scalar(
                        out=rs2m, in0=rs2,
                        scalar1=m01_all[:, 2 * u + lt:2 * u + lt + 1],
                        scalar2=None, op0=ALU.mult)
                    x2 = natp.tile([128, H], BF16, tag="x2")
                    nc.vector.tensor_scalar(
                        out=x2, in0=x2in, scalar1=mean2, scalar2=rs2m,
                        op0=ALU.subtract, op1=ALU.mult)
                    nc.tensor.matmul(pool_ps, x2, ones_b,
                                     start=(lt == 0), stop=(lt == 1))
                nc.any.tensor_copy(pooled[:, u:u + 1], pool_ps)

            # ---- per-group two-phase emission ----
            for g in range(NGRP):
                xs = xpool.tile([DSEQ, GRP * L], BF16, tag="X")
                nc.sync.dma_start(out=xs, in_=x_in[g, :, :])
                states = [phase_a(xs, kk, g * GRP + kk) for kk in range(GRP)]
                for st in states:
                    phase_b(st)

            # ---- per-core tail: unit_fc, building-sum, fusion MLP ----
            u16_ps = psn.tile([UNITD, NU], F32, tag="psn")
            nc.tensor.matmul(u16_ps, w_u, pooled, start=True, stop=True)
            u16 = work.tile([UNITD, NU], F32, tag="u16")
            nc.scalar.activation(out=u16, in_=u16_ps, func=AF.Relu,
                                 bias=0.0, scale=1.0)

            u16t_ps = psn.tile([NU, UNITD], F32, tag="psn")
            nc.tensor.transpose(u16t_ps, u16, ident[:UNITD, :UNITD])
            u16t = work.tile([NU, UNITD], BF16, tag="u16t")
            nc.any.tensor_copy(u16t, u16t_ps)

            seq_ps = psn.tile([UNITD, BPC], F32, tag="psn")
            nc.tensor.matmul(seq_ps, u16t, s_sb, start=True, stop=True)

            fused = work.tile([UNITD + AGGD + TODD, BPC], BF16, tag="fused")
            nc.any.tensor_copy(fused[:UNITD, :], seq_ps)
            nc.gpsimd.dma_start(out=fused[UNITD:, :], in_=tail_in[:, :])

            h1_ps = psn.tile([H, BPC], F32, tag="psn")
            nc.tensor.matmul(h1_ps, w_c1, fused, start=True, stop=True)
            h1 = work.tile([H, BPC], BF16, tag="h1")
            nc.scalar.activation(out=h1, in_=h1_ps, func=AF.Relu,
                                 bias=0.0, scale=1.0)

            o_ps = psn.tile([DOUT, BPC], F32, tag="psn")
            nc.tensor.matmul(o_ps, w_c2, h1, start=True, stop=True)
            o_s = work.tile([DOUT, BPC], F32, tag="osb")
            nc.scalar.activation(out=o_s, in_=o_ps, func=AF.Relu,
                                 bias=0.0, scale=1.0)
            nc.sync.dma_start(out=out_t[:, :], in_=o_s)

    return nc


def _prep_weights(inputs):
    ipw = np.asarray(inputs["in_proj_w"])
    wts = {
        "w_inT": np.asarray(inputs["W_in"]).T,       # [5,128]
        "w_gT": (ipw[0:H] @ ipw[H:2 * H].T),          # Wq^T Wk composed [128,128]
        "w_vT": ipw[2 * H:3 * H].T,
        "w_oT": np.asarray(inputs["out_proj_w"]).T,
        "w_f1T": np.asarray(inputs["W_ff1"]).T,
        "w_f2T": np.asarray(inputs["W_ff2"]).T,
        "w_uT": np.asarray(inputs["W_unit"]).T,       # [128,16]
        "w_c1T": np.asarray(inputs["W_fc1"]).T,       # [26,128]
        "w_c2T": np.asarray(inputs["W_fc2"]).T,       # [128,128]
    }
    wts = {k: np.ascontiguousarray(v.astype(NPBF)) for k, v in wts.items()}
    # the kernel folds no biases / LN affines: assert they are trivial
    for nm in ("b_in", "in_proj_b", "out_proj_b", "b_ff1", "b_ff2",
               "ln1_b", "ln2_b", "b_unit", "b_fc1", "b_fc2"):
        assert np.max(np.abs(np.asarray(inputs[nm]))) == 0.0, f"{nm} nonzero"
    for nm in ("ln1_w", "ln2_w"):
        assert np.allclose(np.asarray(inputs[nm]), 1.0), f"{nm} nontrivial"
    return wts


def make_in_maps(inputs):
    x_seq = np.asarray(inputs["x_seq"], dtype=np.float32)       # [B,U,L,5]
    lengths = np.asarray(inputs["lengths"])                      # [B,U] int
    x_agg = np.asarray(inputs["x_agg_quant"], dtype=np.float32)  # [B,7]
    tod_emb = np.asarray(inputs["tod_emb"], dtype=np.float32)    # [5,3]
    tod_idx = np.asarray(inputs["tod_idx"])                      # [B] int

    in_maps = []
    for c in range(NCORES):
        bs = slice(c * BPC, (c + 1) * BPC)
        xc = x_seq[bs].reshape(NU, L, DSEQ).transpose(0, 2, 1)   # [128,5,256]
        xg = np.ascontiguousarray(
            xc.reshape(NGRP, GRP, DSEQ, L).transpose(0, 2, 1, 3)
            .reshape(NGRP, DSEQ, GRP * L)).astype(NPBF)
        lens = lengths[bs].reshape(NU).astype(np.float32)
        iota = np.arange(L, dtype=np.float32).reshape(2, 128).T  # [128p, 2 tiles]
        # resident mask tile [128p, NU*2]: col 2u+t = (p + 128t) < len[u]
        m01 = (iota[:, None, :] < lens[None, :, None]).astype(np.float32)
        m01 = m01.reshape(128, NU * 2)
        S = np.zeros((NU, BPC), np.float32)
        S[np.arange(NU), np.arange(NU) // U] = 1.0
        tail = np.concatenate(
            [x_agg[bs].T, tod_emb[tod_idx[bs]].T], axis=0)
        in_maps.append({"xg": xg, "m01": np.ascontiguousarray(m01),
                        "m01b": np.ascontiguousarray(m01).astype(NPBF),
                        "S": S.astype(NPBF),
                        "tail": np.ascontiguousarray(tail).astype(NPBF)})
    return in_maps


def kernel(_trace=False, **inputs):
    wts = _prep_weights(inputs)
    nc = build_nc(wts)
    if not nc.is_finalized():
        nc.finalize()
    in_maps = make_in_maps(inputs)
    res = run_bass_kernel_spmd(nc, in_maps, core_ids=list(range(NCORES)),
                               trace=_trace)
    out = np.zeros((B, DOUT), np.float32)
    for c in range(NCORES):
        out[c * BPC:(c + 1) * BPC, :] = res.results[c]["outT"].T
    if _trace:
        kernel._last_results = res
    return out
